# revision 8
# baseline (speedup 1.0000x reference)
"""Trainium2 Bass kernel for the CPG spiking-network problem.

Full inputs in, full outputs out. Data-parallel over the agent batch dim B
across 8 NeuronCores. See build_program() for the device-side design.

Math (per agent, N=32 neurons, E=25, K=16 neighbors, T ticks):
  coupling = sum_k W @ x_k                    [32]   (W: [32,25])
  v' = (2/3) v + (1/6) M @ s + 0.1 coupling   (s = previous spikes)
  s  = [v' >= 1];  v = v' (1 - s);  acc += s
Outputs: E = acc[:25] (f32), should_stay = sum(acc[16:]) > 0.5 (bool).

Device formulation (state transposed: neurons on partitions, agents on free):
  sigma = sign(v - 1) = 2 s - 1 computed on ACT; all sigma consumers are
  linear matmuls, so constants fold:
    (1/6) M @ s = (1/12) M @ sigma + rs,  rs = (1/12) M @ 1
  c'' = 0.1 coupling + rs is the tick-constant drive (rs added as ACT bias).
  v-reset (v * [v<1]) is one fused DVE scalar_tensor_tensor.
  Spike counts accumulate in PSUM via ident@sigma and Tsel@sigma matmuls:
    acc = (sum_t sigma + T) / 2;  should_stay <=> trough_sigma > 1.5 - 16T.
"""

import numpy as np

import concourse.bass as bass
import concourse.bacc as bacc
import concourse.tile as tile
from concourse import mybir
from concourse.bass_utils import run_bass_kernel_spmd

F32 = mybir.dt.float32
U8 = mybir.dt.uint8

N_CORES = 8
B_FULL = 262144
NN = 32            # neurons
NE = 25            # excitatory neurons (output)
NK = 16            # neighbors
PEAK = 16
SLAB = 2048        # agents per slab: 4 blocks x 512 free
BLK = 512          # agents per 32-neuron partition block
GRP = 1024         # agents per X-load group (8 tiles of 128)


def make_consts(M: np.ndarray, W: np.ndarray):
    """Host-side prep of tiny constant matrices fed as extra kernel inputs."""
    M = np.asarray(M, dtype=np.float32)
    W = np.asarray(W, dtype=np.float32)
    mt = (M.T / 12.0).astype(np.float32)                      # [32, 32]
    m4s = np.kron(np.eye(4, dtype=np.float32), mt)            # [128, 128]
    ident = np.eye(128, dtype=np.float32)
    i23 = (2.0 / 3.0) * ident
    w25 = (0.1 * W.T).astype(np.float32)                      # [25, 32]
    wstack = np.zeros((128, 128), dtype=np.float32)
    for j in range(4):
        wstack[32 * j : 32 * j + 25, 32 * j : 32 * j + 32] = w25
    tsel = np.zeros((128, 4), dtype=np.float32)
    for j in range(4):
        tsel[32 * j + PEAK : 32 * j + 32, j] = 1.0
    rs = np.tile((M.sum(axis=1) / 12.0).astype(np.float32), 4).reshape(128, 1)
    return {
        "m4s": m4s,
        "i23": i23,
        "ident": ident,
        "w25": wstack,
        "tsel": tsel,
        "rs": rs,
        "neg1": np.full((128, 1), -1.0, dtype=np.float32),
    }


def build_program(bl: int, t_ticks: int) -> bass.Bass:
    """Build the per-core Bass program for bl agents and t_ticks ticks."""
    assert bl % SLAB == 0
    n_slab = bl // SLAB
    T = t_ticks
    assert T >= 1

    nc = bacc.Bacc("TRN2", target_bir_lowering=False, debug=False)
    x_d = nc.declare_dram_parameter("x", [bl, NK * NE], F32, isOutput=False)
    m4s_d = nc.declare_dram_parameter("m4s", [128, 128], F32, isOutput=False)
    i23_d = nc.declare_dram_parameter("i23", [128, 128], F32, isOutput=False)
    ident_d = nc.declare_dram_parameter("ident", [128, 128], F32, isOutput=False)
    w25_d = nc.declare_dram_parameter("w25", [128, 128], F32, isOutput=False)
    tsel_d = nc.declare_dram_parameter("tsel", [128, 4], F32, isOutput=False)
    rs_d = nc.declare_dram_parameter("rs", [128, 1], F32, isOutput=False)
    neg1_d = nc.declare_dram_parameter("neg1", [128, 1], F32, isOutput=False)
    thalf_d = nc.declare_dram_parameter("thalf", [128, 1], F32, isOutput=False)
    e_d = nc.declare_dram_parameter("e_out", [bl, NE], F32, isOutput=True)
    ss_d = nc.declare_dram_parameter("ss_out", [bl], U8, isOutput=True)

    # DRAM views for the stores
    # e_out rows grouped (slab, block j, colgrp c, agent a) -> iterate (a, j, e)
    e_v = e_d[:].rearrange("(s j c a) e -> s c a j e", j=4, c=4, a=128)
    ss_v = ss_d[:].rearrange("(s j f) -> s j f", j=4, f=BLK)
    x_v = x_d[:].rearrange("(g t p) c -> g p t c", t=GRP // 128, p=128)

    with tile.TileContext(nc) as tc:
        with (
            tc.tile_pool(name="consts", bufs=1) as cpool,
            tc.tile_pool(name="xload", bufs=3) as xpool,
            tc.tile_pool(name="half", bufs=2) as hpool,
            tc.tile_pool(name="ssum", bufs=2) as spool,
            tc.tile_pool(name="stsb", bufs=4) as stpool,
            tc.tile_pool(name="state", bufs=2) as vpool,
            tc.tile_pool(name="sig", bufs=2) as sgpool,
            tc.tile_pool(name="cdd", bufs=2) as cpool2,
            tc.tile_pool(name="eout", bufs=2) as epool,
            tc.tile_pool(name="outsb", bufs=4) as opool,
            tc.tile_pool(name="sssb", bufs=2) as sspool,
            tc.tile_pool(name="pv", bufs=2, space=bass.MemorySpace.PSUM) as pv,
            tc.tile_pool(name="pst", bufs=4, space=bass.MemorySpace.PSUM) as pst,
            tc.tile_pool(name="pmacc", bufs=1, space=bass.MemorySpace.PSUM) as pmacc,
            tc.tile_pool(name="ptr", bufs=1, space=bass.MemorySpace.PSUM) as ptr,
        ):
            # --- constants to SBUF ---
            m4s = cpool.tile([128, 128], F32)
            nc.sync.dma_start(m4s[:], m4s_d[:])
            i23 = cpool.tile([128, 128], F32)
            nc.sync.dma_start(i23[:], i23_d[:])
            ident = cpool.tile([128, 128], F32)
            nc.sync.dma_start(ident[:], ident_d[:])
            w25 = cpool.tile([128, 128], F32)
            nc.sync.dma_start(w25[:], w25_d[:])
            tsel = cpool.tile([128, 4], F32)
            nc.sync.dma_start(tsel[:], tsel_d[:])
            rs = cpool.tile([128, 1], F32)
            nc.sync.dma_start(rs[:], rs_d[:])
            neg1 = cpool.tile([128, 1], F32)
            nc.sync.dma_start(neg1[:], neg1_d[:])
            thalf = cpool.tile([128, 1], F32)
            nc.sync.dma_start(thalf[:], thalf_d[:])

            for sb in range(n_slab):
                # ---- stage A: load + k-sum + transpose + coupling ----
                s_tiles = []
                for g in range(2):
                    xg = xpool.tile([128, 8, NK * NE], F32, tag="xg")
                    nc.sync.dma_start(xg[:], x_v[sb * 2 + g])
                    hg = hpool.tile([128, 8, 8 * NE], F32, tag="hg")
                    nc.gpsimd.tensor_add(
                        hg[:], xg[:, :, : 8 * NE], xg[:, :, 8 * NE :]
                    )
                    sg = spool.tile([128, 8, 32], F32, tag="sg")
                    nc.gpsimd.memset(sg[:], 0.0)
                    # reduce over k (stride NE) as innermost AP axis
                    hview = hg[:].rearrange("p t (k e) -> p t e k", k=8)
                    nc.vector.tensor_reduce(
                        sg[:, :, :NE], hview, axis=mybir.AxisListType.X,
                        op=mybir.AluOpType.add,
                    )
                    s_tiles.append(sg)

                vp0 = pv.tile([128, BLK], F32, tag="vp")
                stsb = stpool.tile([128, BLK], F32, tag="stsb")
                for j in range(4):
                    stp = pst.tile([32, BLK], F32, tag="stp")
                    for cgi in range(4):
                        ti = j * 4 + cgi          # tile index in slab (0..15)
                        sg = s_tiles[ti // 8]
                        nc.tensor.matmul(
                            stp[:, cgi * 128 : (cgi + 1) * 128],
                            sg[:, ti % 8, :],     # lhsT [128, 32]
                            ident[:],             # rhs  [128, 128]
                            start=True, stop=True,
                        )
                    nc.scalar.copy(stsb[32 * j : 32 * j + 32, :], stp[:])
                # coupling*0.1 all 4 blocks in one block-diag matmul
                nc.tensor.matmul(
                    vp0[:], w25[:], stsb[:], start=True, stop=True,
                )

                # tick-constant drive c'' = 0.1*coupling + rs
                cdd = cpool2.tile([128, BLK], F32, tag="cdd")
                nc.scalar.activation(
                    cdd[:], vp0[:], mybir.ActivationFunctionType.Identity,
                    bias=rs[:], scale=1.0,
                )

                # ---- ticks ----
                macc = pmacc.tile([128, BLK], F32, tag="macc")
                trough = ptr.tile([4, BLK], F32, tag="trough")
                vp = vp0
                for t in range(1, T + 1):
                    sig = sgpool.tile([128, BLK], F32, tag="sig")
                    nc.scalar.activation(
                        sig[:], vp[:], mybir.ActivationFunctionType.Sign,
                        bias=neg1[:], scale=1.0,
                    )
                    nc.tensor.matmul(
                        macc[:], ident[:], sig[:],
                        start=(t == 1), stop=(t == T),
                    )
                    nc.tensor.matmul(
                        trough[:], tsel[:], sig[:],
                        start=(t == 1), stop=(t == T),
                    )
                    if t < T:
                        vsb = vpool.tile([128, BLK], F32, tag="vsb")
                        # v-reset: (sigma<0) selects non-spiking; vp read once
                        nc.vector.scalar_tensor_tensor(
                            vsb[:], sig[:], 0.0, vp[:],
                            op0=mybir.AluOpType.is_lt,
                            op1=mybir.AluOpType.mult,
                        )
                        vp_new = pv.tile([128, BLK], F32, tag="vp")
                        nc.tensor.matmul(vp_new[:], i23[:], vsb[:],
                                         start=True, stop=False)
                        nc.tensor.matmul(vp_new[:], m4s[:], sig[:],
                                         start=False, stop=False)
                        nc.tensor.matmul(vp_new[:], ident[:], cdd[:],
                                         start=False, stop=True)
                        vp = vp_new

                # ---- outputs ----
                esb = epool.tile([128, BLK], F32, tag="esb")
                nc.scalar.activation(
                    esb[:], macc[:], mybir.ActivationFunctionType.Identity,
                    bias=thalf[:], scale=0.5,
                )
                for g4 in range(4):
                    tp = pst.tile([128, 128], F32, tag="stp")
                    nc.tensor.matmul(
                        tp[:], esb[:, g4 * 128 : (g4 + 1) * 128], ident[:],
                        start=True, stop=True,
                    )
                    osb = opool.tile([128, 128], F32, tag="osb")
                    nc.scalar.copy(osb[:], tp[:])
                    src = osb[:].rearrange("p (j n) -> p j n", j=4)[:, :, :NE]
                    nc.sync.dma_start(e_v[sb, g4], src)

                sssb = sspool.tile([4, BLK], U8, tag="sssb")
                nc.vector.tensor_scalar(
                    sssb[:], trough[:], float(1.5 - 16 * T), None,
                    op0=mybir.AluOpType.is_gt,
                )
                nc.sync.dma_start(ss_v[sb], sssb[:])

    nc.compile()
    return nc


_CACHE: dict = {}
TRACE = False          # test harness sets True to collect HW exec time
LAST_RESULTS = None    # BassKernelResults of the most recent run


def _get_program(bl: int, t: int) -> bass.Bass:
    key = (bl, t)
    if key not in _CACHE:
        _CACHE[key] = build_program(bl, t)
    return _CACHE[key]


def kernel(neighbor_E_spikes, mutual_inhibition, coupling_weight, num_ticks):
    x = np.asarray(neighbor_E_spikes)
    M = np.asarray(mutual_inhibition, dtype=np.float32)
    W = np.asarray(coupling_weight, dtype=np.float32)
    T = int(num_ticks)
    B = x.shape[0]

    if T <= 0:
        return (
            np.zeros((B, NE), dtype=np.float32),
            np.zeros((B,), dtype=bool),
        )

    assert B % N_CORES == 0
    bl = B // N_CORES
    consts = make_consts(M, W)
    x2 = np.ascontiguousarray(x, dtype=np.float32).reshape(B, NK * NE)

    nc = _get_program(bl, T)
    in_maps = []
    for c in range(N_CORES):
        m = {"x": x2[c * bl : (c + 1) * bl],
             "thalf": np.full((128, 1), T / 2.0, dtype=np.float32)}
        m.update(consts)
        in_maps.append(m)

    global LAST_RESULTS
    LAST_RESULTS = run_bass_kernel_spmd(
        nc, in_maps, list(range(N_CORES)), trace=TRACE
    )
    res = LAST_RESULTS.results
    e = np.concatenate([res[c]["e_out"] for c in range(N_CORES)], axis=0)
    ss = np.concatenate([res[c]["ss_out"] for c in range(N_CORES)], axis=0)
    return e.astype(np.float32, copy=False), ss.view(np.bool_)


# revision 14
# speedup vs baseline: 198.0097x; 198.0097x over previous
"""Trainium2 Bass kernel for the CPG spiking-network problem.

Full inputs in, full outputs out. Data-parallel over the agent batch dim B
across 8 NeuronCores. See build_program() for the device-side design.

Math (per agent, N=32 neurons, E=25, K=16 neighbors, T ticks):
  coupling = sum_k W @ x_k                    [32]   (W: [32,25])
  v' = (2/3) v + (1/6) M @ s + 0.1 coupling   (s = previous spikes)
  s  = [v' >= 1];  v = v' (1 - s);  acc += s
Outputs: E = acc[:25] (f32), should_stay = sum(acc[16:]) > 0.5 (bool).

Device formulation (state transposed: neurons on partitions, agents on free):
  sigma = sign(v - 1) = 2 s - 1 computed on ACT; all sigma consumers are
  linear matmuls, so constants fold:
    (1/6) M @ s = (1/12) M @ sigma + rs,  rs = (1/12) M @ 1
  c'' = 0.1 coupling + rs is the tick-constant drive (rs added as ACT bias).
  v-reset (v * [v<1]) is one fused DVE scalar_tensor_tensor.
  Spike counts accumulate in PSUM via ident@sigma and Tsel@sigma matmuls:
    acc = (sum_t sigma + T) / 2;  should_stay <=> trough_sigma > 1.5 - 16T.
"""

import numpy as np

import concourse.bass as bass
import concourse.bacc as bacc
import concourse.tile as tile
from concourse import mybir
from concourse.bass_utils import run_bass_kernel_spmd

F32 = mybir.dt.float32
U8 = mybir.dt.uint8

N_CORES = 8
B_FULL = 262144
NN = 32            # neurons
NE = 25            # excitatory neurons (output)
NK = 16            # neighbors
PEAK = 16
SLAB = 2048        # agents per slab: 4 blocks x 512 free
BLK = 512          # agents per 32-neuron partition block
import os
G_TILES = int(os.environ.get("KQ_GTILES", "4"))   # tiles per X-load group
PV_BUFS = int(os.environ.get("KQ_PV", "4"))


def make_consts(M: np.ndarray, W: np.ndarray):
    """Host-side prep of tiny constant matrices fed as extra kernel inputs."""
    M = np.asarray(M, dtype=np.float32)
    W = np.asarray(W, dtype=np.float32)
    mt = (M.T / 12.0).astype(np.float32)                      # [32, 32]
    m4s = np.kron(np.eye(4, dtype=np.float32), mt)            # [128, 128]
    ident = np.eye(128, dtype=np.float32)
    i23 = (2.0 / 3.0) * ident
    w25 = (0.1 * W.T).astype(np.float32)                      # [25, 32]
    wstack = np.zeros((128, 128), dtype=np.float32)
    for j in range(4):
        wstack[32 * j : 32 * j + 25, 32 * j : 32 * j + 32] = w25
    rs = np.tile((M.sum(axis=1) / 12.0).astype(np.float32), 4).reshape(128, 1)
    return {
        "m4s": m4s,
        "i23": i23,
        "ident": ident,
        "w25": wstack,
        "rs": rs,
        "neg1": np.full((128, 1), -1.0, dtype=np.float32),
    }


def build_program(bl: int, t_ticks: int) -> bass.Bass:
    """Build the per-core Bass program for bl agents and t_ticks ticks."""
    assert bl % SLAB == 0
    n_slab = bl // SLAB
    T = t_ticks
    assert T >= 1

    nc = bacc.Bacc("TRN2", target_bir_lowering=False, debug=False)
    x_d = nc.declare_dram_parameter("x", [bl, NK * NE], F32, isOutput=False)
    m4s_d = nc.declare_dram_parameter("m4s", [128, 128], F32, isOutput=False)
    i23_d = nc.declare_dram_parameter("i23", [128, 128], F32, isOutput=False)
    ident_d = nc.declare_dram_parameter("ident", [128, 128], F32, isOutput=False)
    w25_d = nc.declare_dram_parameter("w25", [128, 128], F32, isOutput=False)
    rs_d = nc.declare_dram_parameter("rs", [128, 1], F32, isOutput=False)
    neg1_d = nc.declare_dram_parameter("neg1", [128, 1], F32, isOutput=False)
    thalf_d = nc.declare_dram_parameter("thalf", [128, 1], F32, isOutput=False)
    e_d = nc.declare_dram_parameter("e_out", [bl, NE], F32, isOutput=True)
    ss_d = nc.declare_dram_parameter("ss_out", [bl], U8, isOutput=True)

    # DRAM views for the stores
    # e_out rows grouped (slab, block j, colgrp c, agent a) -> iterate (a, j, e)
    e_v = e_d[:].rearrange("(s c j a) e -> s a c j e", j=4, c=4, a=128)
    ss_v2 = ss_d[:].rearrange("(s c j a) -> s a c j", j=4, c=4, a=128)
    x_v = x_d[:].rearrange("(g t p) c -> g p t c", t=G_TILES, p=128)

    with tile.TileContext(nc) as tc:
        with (
            tc.tile_pool(name="consts", bufs=1) as cpool,
            tc.tile_pool(name="xload", bufs=3) as xpool,
            tc.tile_pool(name="half", bufs=2) as hpool,
            tc.tile_pool(name="ssum", bufs=2) as spool,
            tc.tile_pool(name="stsb", bufs=4) as stpool,
            tc.tile_pool(name="state", bufs=2) as vpool,
            tc.tile_pool(name="sig", bufs=2) as sgpool,
            tc.tile_pool(name="cdd", bufs=2) as cpool2,
            tc.tile_pool(name="eout", bufs=2) as epool,
            tc.tile_pool(name="outsb", bufs=4) as opool,
            tc.tile_pool(name="sssb", bufs=2) as sspool,
            tc.tile_pool(name="pv", bufs=PV_BUFS, space=bass.MemorySpace.PSUM) as pv,
            tc.tile_pool(name="pst", bufs=2, space=bass.MemorySpace.PSUM) as pst,
            tc.tile_pool(name="pmacc", bufs=2, space=bass.MemorySpace.PSUM) as pmacc,
        ):
            # --- constants to SBUF ---
            m4s = cpool.tile([128, 128], F32)
            nc.sync.dma_start(m4s[:], m4s_d[:])
            i23 = cpool.tile([128, 128], F32)
            nc.sync.dma_start(i23[:], i23_d[:])
            ident = cpool.tile([128, 128], F32)
            nc.sync.dma_start(ident[:], ident_d[:])
            w25 = cpool.tile([128, 128], F32)
            nc.sync.dma_start(w25[:], w25_d[:])
            rs = cpool.tile([128, 1], F32)
            nc.sync.dma_start(rs[:], rs_d[:])
            neg1 = cpool.tile([128, 1], F32)
            nc.sync.dma_start(neg1[:], neg1_d[:])
            thalf = cpool.tile([128, 1], F32)
            nc.sync.dma_start(thalf[:], thalf_d[:])

            for sb in range(n_slab):
                # ---- stage A: load + k-sum + transpose + coupling ----
                ngr = 16 // G_TILES
                sg = spool.tile([128, 16, 32], F32, tag="sg")
                nc.gpsimd.memset(sg[:], 0.0)
                for g in range(ngr):
                    xg = xpool.tile([128, G_TILES, NK * NE], F32, tag="xg")
                    nc.sync.dma_start(xg[:], x_v[sb * ngr + g])
                    hg = hpool.tile([128, G_TILES, 8 * NE], F32, tag="hg")
                    nc.gpsimd.tensor_add(
                        hg[:], xg[:, :, : 8 * NE], xg[:, :, 8 * NE :]
                    )
                    # reduce over k (stride NE) as innermost AP axis
                    hview = hg[:].rearrange("p t (k e) -> p t e k", k=8)
                    nc.vector.tensor_reduce(
                        sg[:, g * G_TILES : (g + 1) * G_TILES, :NE],
                        hview, axis=mybir.AxisListType.X,
                        op=mybir.AluOpType.add,
                    )

                vp0 = pv.tile([128, BLK], F32, tag="vp")
                stsb = stpool.tile([128, BLK], F32, tag="stsb")
                for c in range(4):
                    stp = pst.tile([128, 128], F32, tag="stp")
                    # group c tiles {4c..4c+3} stacked: out rows 32j hold
                    # tile (4c+j).T — block j at col-range c (j = ti mod 4)
                    nc.tensor.matmul(
                        stp[:], sg[:, 4 * c : 4 * c + 4, :], ident[:],
                        start=True, stop=True,
                    )
                    nc.scalar.copy(stsb[:, c * 128 : (c + 1) * 128], stp[:])
                # coupling*0.1 all 4 blocks in one block-diag matmul
                nc.tensor.matmul(
                    vp0[:], w25[:], stsb[:], start=True, stop=True,
                )

                # tick-constant drive c'' = 0.1*coupling + rs
                cdd = cpool2.tile([128, BLK], F32, tag="cdd")
                nc.scalar.activation(
                    cdd[:], vp0[:], mybir.ActivationFunctionType.Identity,
                    bias=rs[:], scale=1.0,
                )

                # ---- ticks ----
                macc = pmacc.tile([128, BLK], F32, tag="macc")
                vp = vp0
                for t in range(1, T + 1):
                    sig = sgpool.tile([128, BLK], F32, tag="sig")
                    nc.scalar.activation(
                        sig[:], vp[:], mybir.ActivationFunctionType.Sign,
                        bias=neg1[:], scale=1.0,
                    )
                    nc.tensor.matmul(
                        macc[:], ident[:], sig[:],
                        start=(t == 1), stop=(t == T),
                    )
                    if t < T:
                        vsb = vpool.tile([128, BLK], F32, tag="vsb")
                        # v-reset: (sigma<0) selects non-spiking; vp read once
                        nc.vector.scalar_tensor_tensor(
                            vsb[:], sig[:], 0.0, vp[:],
                            op0=mybir.AluOpType.is_lt,
                            op1=mybir.AluOpType.mult,
                        )
                        vp_new = pv.tile([128, BLK], F32, tag="vp")
                        nc.tensor.matmul(vp_new[:], ident[:], cdd[:],
                                         start=True, stop=False)
                        nc.tensor.matmul(vp_new[:], m4s[:], sig[:],
                                         start=False, stop=False)
                        nc.tensor.matmul(vp_new[:], i23[:], vsb[:],
                                         start=False, stop=True)
                        vp = vp_new

                # ---- outputs ----
                esb = epool.tile([128, BLK], F32, tag="esb")
                nc.scalar.activation(
                    esb[:], macc[:], mybir.ActivationFunctionType.Identity,
                    bias=thalf[:], scale=0.5,
                )
                osb = opool.tile([128, 4, 4, 32], F32, tag="osb")
                ssb = sspool.tile([128, 4, 4], U8, tag="ssb")
                # osb axes: [agent-lane, colgrp g, block j, neuron]
                for g4 in range(4):
                    tp = pst.tile([128, 128], F32, tag="stp")
                    nc.tensor.matmul(
                        tp[:], esb[:, g4 * 128 : (g4 + 1) * 128], ident[:],
                        start=True, stop=True,
                    )
                    tpv = tp[:].rearrange("p (j n) -> p j n", j=4)
                    nc.scalar.copy(osb[:, g4, :, :], tpv)
                    # should_stay: sum trough-neuron acc per agent row
                    trred = sspool.tile([128, 4], F32, tag="trred")
                    tview = osb[:, g4, :, PEAK:]
                    nc.vector.tensor_reduce(
                        trred[:], tview, axis=mybir.AxisListType.X,
                        op=mybir.AluOpType.add,
                    )
                    nc.vector.tensor_scalar(
                        ssb[:, g4, :], trred[:], 0.5, None,
                        op0=mybir.AluOpType.is_gt,
                    )
                nc.sync.dma_start(e_v[sb], osb[:, :, :, :NE])
                nc.sync.dma_start(ss_v2[sb], ssb[:])

    nc.compile()
    return nc


_CACHE: dict = {}
TRACE = False          # test harness sets True to collect HW exec time
LAST_RESULTS = None    # BassKernelResults of the most recent run


def _get_program(bl: int, t: int) -> bass.Bass:
    key = (bl, t)
    if key not in _CACHE:
        _CACHE[key] = build_program(bl, t)
    return _CACHE[key]


def kernel(neighbor_E_spikes, mutual_inhibition, coupling_weight, num_ticks):
    x = np.asarray(neighbor_E_spikes)
    M = np.asarray(mutual_inhibition, dtype=np.float32)
    W = np.asarray(coupling_weight, dtype=np.float32)
    T = int(num_ticks)
    B = x.shape[0]

    if T <= 0:
        return (
            np.zeros((B, NE), dtype=np.float32),
            np.zeros((B,), dtype=bool),
        )

    assert B % N_CORES == 0
    bl = B // N_CORES
    consts = make_consts(M, W)
    x2 = np.ascontiguousarray(x, dtype=np.float32).reshape(B, NK * NE)

    nc = _get_program(bl, T)
    in_maps = []
    for c in range(N_CORES):
        m = {"x": x2[c * bl : (c + 1) * bl],
             "thalf": np.full((128, 1), T / 2.0, dtype=np.float32)}
        m.update(consts)
        in_maps.append(m)

    global LAST_RESULTS
    LAST_RESULTS = run_bass_kernel_spmd(
        nc, in_maps, list(range(N_CORES)), trace=TRACE
    )
    res = LAST_RESULTS.results
    e = np.concatenate([res[c]["e_out"] for c in range(N_CORES)], axis=0)
    ss = np.concatenate([res[c]["ss_out"] for c in range(N_CORES)], axis=0)
    return e.astype(np.float32, copy=False), ss.view(np.bool_)
